# revision 32
# baseline (speedup 1.0000x reference)
"""Trainium2 Bass kernel for nn_AttentionModel (patch-transformer + MSE loss).

Model (per batch element b of B=32):
    x[b] : [L=32768] --instance-norm--> patches [T=1024, PS=32]
    h    = patches @ W_proj + b_proj                  [T, 256]
    qkv  = h @ W_qkv + b_qkv ;  q,k,v = split(qkv)    [T, 256] each
    attn = softmax(causal(q k^T / 16))                [T, T]
    out  = (attn @ v) @ W_out + b_out                 [T, 256]
    pred = out @ W_head + b_head                      [T, PS]
    loss = mean((pred[:, :-1] - patches[:, 1:])**2)   scalar

Sharding: data-parallel over batch, 4 batch elements per core x 8 cores.
Each core computes a partial sum-of-squares; host combines.

v2.1 design (numpy prototype: proto_v2.py):
  - logits factor through patch space: scores[t,s] = x_t^T A x_s + g_s with
    A = Wq_eff Wk_eff^T [32,32]; the per-t bias constant is dropped
    (softmax-invariant), the per-s part g_s = (Wk b_q)^T x_s rides as a
    per-partition bias of the exp.
  - X = patches^T [32, T] bf16 normalized; Y = A^T X once [32, T];
    scores^T[s,t] computed per s-tile j as ONE wide matmul (N<=1024 bf16),
    causally trimmed.
  - exp is split between ScalarE (ACT exp, per-partition bias) and DVE
    (int16 bit-trick exp writing bf16 bits; its systematic error cancels
    in the softmax ratio -- validated 4.5e-6 rel err in proto).
  - PV runs TRANSPOSED: puT[tc] = sum_j et(j,tc)^T vw_j with
    vw = [X^T M_v + 1 c_v^T | ones], so the softmax denominator lands
    per-PARTITION (puT col 32) and normalize+loss is all per-partition:
    reciprocal [128,8] -> fused (puT*r - xn2) scalar_tensor_tensor ->
    Square+accum loss.
  - the shifted target patches xn2 (= patch[t+1] token-major) come from a
    second strided DMA of x (no transposes, no partition shifts).
  - NOTE row-tiled matmuls (tile_position with row base != 0) pass
    compile+sim but hang this HW stack when >1 issue back-to-back; only
    plain/col tiling is used.
"""

import math
import os

import numpy as np

import concourse.bass as bass
import concourse.mybir as mybir
import concourse.tile as tile
from concourse.bass_utils import run_bass_kernel_spmd
from concourse.masks import make_identity, make_upper_triangular
from concourse.vector_clock import ScopedClock

F32 = mybir.dt.float32
BF16 = mybir.dt.bfloat16
I16 = mybir.dt.int16
AX = mybir.AxisListType
ALU = mybir.AluOpType
AF = mybir.ActivationFunctionType

N_CORES = 8
B = 32
L = 32768
PS = 32
D = 256
T = L // PS  # 1024
BPC = B // N_CORES  # batch elements per core = 4
SCALE = 1.0 / math.sqrt(D)  # 1/16
K1 = 128.0 / math.log(2.0)  # bf16 exponent scale for bit-trick exp
B0 = 127.0 * 128.0 - 0.0579 * 128.0  # bias incl. mantissa-centering corr

# j-tiles whose exp runs on DVE (bit-trick); rest on ScalarE ACT exp.
DVE_EXP = {0, 1}


class SplitDrainTileContext(tile.TileContext):
    """TileContext whose final drain splits sem waits across multiple drain
    instructions -- this walrus rejects >1 sync wait per instruction."""

    def _drain_and_barrier(self, tick_clock, wait_clock):
        probe = mybir.InstDrain(name=f"I-{self.nc.next_id()}", ins=[], outs=[])
        probe.engine = mybir.EngineType.SP
        wait_clock.add_sem_waits(probe, ScopedClock({None: tick_clock.global_clock}))
        waits = list(probe.sync_info.on_wait) if probe.sync_info else []
        assert self.sems is not None
        handles = {h.num: h for h in self.sems.allocated().values()}
        if not waits:
            self.nc.sync.drain()
        for w in waits:
            d = self.nc.sync.drain()
            d.wait_op(handles[w.id], w.wait_value, "sem-ge", check=False)
        self.nc.all_engine_barrier()
        popped = self.nc._tile_sem_poison_stack.pop()
        assert popped is self._sem_poison
        self.nc.clear_and_free_semaphores(list(self.sems.allocated().values()))
        self.nc.all_engine_barrier()


def split_excess_waits(nc, max_waits=1):
    """This walrus rejects instructions carrying more than one sync wait.
    Hoist extra waits onto the immediately preceding same-engine
    instruction when that instruction signals nothing, else insert a
    wait-only drain."""
    for f in nc.m.functions:
        for blk in f.blocks:
            insts = list(blk.instructions)
            out = []
            prev_by_engine = {}
            changed = False
            for inst in insts:
                si = inst.sync_info
                waits = list(si.on_wait) if si else []
                if len(waits) > max_waits:
                    changed = True
                    extra, keep = waits[:-max_waits], waits[-max_waits:]
                    remaining = []
                    prev = prev_by_engine.get(str(inst.engine))
                    for w in extra:
                        psi = prev.sync_info if prev is not None else None
                        if prev is not None and (
                            psi is None
                            or (len(psi.on_wait) == 0 and len(psi.on_update) == 0)
                        ):
                            prev.sync_info = mybir.SyncInfo(on_wait=[w], on_update=[])
                            prev = None  # one hoist per predecessor
                        else:
                            remaining.append(w)
                    for w in remaining:
                        dr = mybir.InstDrain(name=f"I-{nc.next_id()}", ins=[], outs=[])
                        dr.engine = inst.engine
                        dr.sync_info = mybir.SyncInfo(on_wait=[w], on_update=[])
                        out.append(dr)
                    inst.sync_info = mybir.SyncInfo(
                        on_wait=keep, on_update=list(si.on_update)
                    )
                out.append(inst)
                prev_by_engine[str(inst.engine)] = inst
            if changed:
                blk.instructions = out


def dedupe_ldweights(nc):
    """Drop an InstLdweights whose operand AP AND tile_position are
    byte-identical to the immediately preceding PE instruction's
    InstLdweights -- the stationary operand is still loaded. Only legal
    when the elided load carries no sync actions."""
    for f in nc.m.functions:
        for blk in f.blocks:
            insts = list(blk.instructions)
            out = []
            last_pe_ldw_key = None
            changed = False
            for inst in insts:
                if str(inst.engine) != "EngineType.PE":
                    out.append(inst)
                    continue
                tname = type(inst).__name__
                if tname == "InstLdweights":
                    si = inst.sync_info
                    has_sync = si and (len(si.on_wait) or len(si.on_update))
                    try:
                        key = (
                            str(inst.ins[0]),
                            str(getattr(inst, "tile_position", None)),
                        )
                    except Exception:
                        key = None
                    if key is not None and key == last_pe_ldw_key and not has_sync:
                        changed = True
                        continue  # elide duplicate load
                    last_pe_ldw_key = key
                    out.append(inst)
                else:
                    if tname == "InstMatmult":
                        # transpose-mode matmuls reload the array themselves
                        if getattr(inst, "is_transpose", None):
                            last_pe_ldw_key = None
                    else:
                        last_pe_ldw_key = None
                    out.append(inst)
            if changed:
                blk.instructions = out


def build_program(postprocess=True, has_bias=False):
    nc = bass.Bass("TRN2", target_bir_lowering=False, debug=False, num_devices=N_CORES)

    x_d = nc.dram_tensor("x", [BPC, L], F32, kind="ExternalInput")
    a_d = nc.dram_tensor("a_mat", [PS, PS], BF16, kind="ExternalInput")
    mvg_d = nc.dram_tensor("mvg", [PS, PS + 1], BF16, kind="ExternalInput")
    cv_d = nc.dram_tensor("cv_rep", [128, PS], F32, kind="ExternalInput")
    wkb_d = nc.dram_tensor("wkb", [PS, 1], BF16, kind="ExternalInput")
    out_d = nc.dram_tensor("loss_partial", [1, 1], F32, kind="ExternalOutput")

    from contextlib import ExitStack

    with SplitDrainTileContext(nc) as tc, ExitStack() as ctx:
        cpool = ctx.enter_context(tc.tile_pool(name="consts", bufs=1))
        # PSUM pools (8 banks x 2KB/partition): pscore 3x2 banks + ppu 2x1
        pscore = ctx.enter_context(tc.tile_pool(name="pscore", bufs=2, space="PSUM"))
        pxty = ctx.enter_context(tc.tile_pool(name="pxty", bufs=2, space="PSUM"))
        ppu = ctx.enter_context(tc.tile_pool(name="ppu", bufs=2, space="PSUM"))
        # SBUF pools
        xpool = ctx.enter_context(tc.tile_pool(name="xc", bufs=4))
        x2pool = ctx.enter_context(tc.tile_pool(name="xc2", bufs=4))
        qpool = ctx.enter_context(tc.tile_pool(name="xnt", bufs=4))
        ypool = ctx.enter_context(tc.tile_pool(name="y", bufs=3))
        vpool = ctx.enter_context(tc.tile_pool(name="vw", bufs=3))
        epool = ctx.enter_context(tc.tile_pool(name="et", bufs=3))
        spool = ctx.enter_context(tc.tile_pool(name="small", bufs=6))
        lpool = ctx.enter_context(tc.tile_pool(name="loss", bufs=4))

        # ---- constants ----
        ident_f = cpool.tile([128, 128], F32)
        make_identity(nc, ident_f[:])
        triu_b = cpool.tile([128, 128], BF16)
        make_upper_triangular(nc, triu_b[:], val=1.0, diag=True)
        ones_col = cpool.tile([128, 1], F32)
        nc.vector.memset(ones_col[:], 1.0)
        ones_row = cpool.tile([1, 128], F32)
        nc.vector.memset(ones_row[:], 1.0)

        a_mat = cpool.tile([PS, PS], BF16)
        nc.gpsimd.dma_start(a_mat[:], a_d.ap()[:])
        mvg = cpool.tile([PS, PS + 1], BF16)
        nc.gpsimd.dma_start(mvg[:], mvg_d.ap()[:])
        cv_rep = cpool.tile([128, PS], F32)
        nc.gpsimd.dma_start(cv_rep[:], cv_d.ap()[:])
        wkb = cpool.tile([PS, 1], BF16)
        nc.gpsimd.dma_start(wkb[:], wkb_d.ap()[:])

        lp = cpool.tile([128, BPC], F32)  # per-batch loss partials
        # last-partition mask: zeroes the t=1023 dd column (excluded by
        # pred[:, :-1]; its xn2 is garbage)
        lmask = cpool.tile([128, 1], F32)
        nc.vector.tensor_scalar(
            out=lmask[:],
            in0=ident_f[:, 127:128],
            scalar1=-1.0,
            scalar2=1.0,
            op0=ALU.mult,
            op1=ALU.add,
        )

        # PE warm-up: HAM holds PE at 1.2 GHz until ~3.4us sustained
        # activity; burn dummy matmuls while the DMAs + stats run.
        warm_ps = pscore.tile([128, 1024], F32, tag="sc")
        for _ in range(42):
            nc.tensor.matmul(
                warm_ps[:, 0:128], triu_b[:], triu_b[:], start=True, stop=True
            )

        # ---- load x and x-shifted-by-one-patch; stats for all batches ----
        xc = []
        xc2 = []
        for b in range(BPC):
            t_ = xpool.tile([128, L // 128], F32, name=f"xc{b}", tag="xc")
            # partition u, free (k, ps) <- x[b, (128k+u)*32 + ps]
            (nc.sync if b % 2 == 0 else nc.scalar).dma_start(
                t_[:].rearrange("u (k ps) -> u k ps", ps=PS),
                x_d.ap()[b].rearrange("(k u ps) -> u k ps", u=128, ps=PS),
            )
            xc.append(t_)
            t2 = x2pool.tile([128, L // 128], F32, name=f"xc2{b}", tag="xc2")
            # the DMA leaves [127, 224:256] unwritten (no patch 1024); zero it
            # first so the masked-out t=1023 lane can't be NaN garbage
            nc.gpsimd.memset(t2[:, 7 * PS : 8 * PS], 0.0)
            # token-major target patches, shifted one patch
            nc.gpsimd.dma_start(
                t2[:, 0 : 7 * PS].rearrange("u (c p) -> u c p", p=PS),
                x_d.ap()[b, PS : PS + 7 * 128 * PS].rearrange(
                    "(c u p) -> u c p", u=128, p=PS
                ),
            )
            nc.gpsimd.dma_start(
                t2[0:127, 7 * PS : 8 * PS],
                x_d.ap()[b, PS + 7 * 128 * PS : L].rearrange("(u p) -> u p", p=PS),
            )
            xc2.append(t2)

        sums = spool.tile([128, 2 * BPC], F32, name="sums")
        sqscr = spool.tile([128, L // 128], F32, name="sqscr")
        for b in range(BPC):
            nc.vector.tensor_reduce(
                sums[:, b : b + 1], xc[b][:], axis=AX.X, op=ALU.add
            )
            nc.scalar.activation(
                sqscr[:],
                xc[b][:],
                AF.Square,
                accum_out=sums[:, BPC + b : BPC + b + 1],
            )
        tot_ps = ppu.tile([1, 2 * BPC], F32, tag="pu")
        nc.tensor.matmul(tot_ps[:], ones_col[:], sums[:], start=True, stop=True)
        tot = spool.tile([1, 2 * BPC], F32, name="tot")
        nc.vector.tensor_copy(tot[:], tot_ps[:])

        # vectorized stats chain over the 4 batches
        sc = spool.tile([1, 6 * BPC], F32, name="sc")
        mean = sc[:, 0:BPC]
        sm = sc[:, BPC : 2 * BPC]
        varr = sc[:, 2 * BPC : 3 * BPC]
        lnv = sc[:, 3 * BPC : 4 * BPC]
        stde = sc[:, 4 * BPC : 5 * BPC]
        mr = sc[:, 5 * BPC : 6 * BPC]
        scv = spool.tile([1, 2 * BPC], F32, name="scv")  # [rstd x4, -m*rstd x4]
        nc.scalar.mul(mean, tot[:, 0:BPC], 1.0 / L)
        nc.vector.tensor_tensor(out=sm, in0=tot[:, 0:BPC], in1=mean, op=ALU.mult)
        nc.vector.tensor_tensor(
            out=varr, in0=tot[:, BPC : 2 * BPC], in1=sm, op=ALU.subtract
        )
        nc.scalar.activation(lnv, varr, AF.Ln, scale=1.0 / (L - 1))
        nc.scalar.activation(stde, lnv, AF.Exp, scale=0.5)  # std
        nc.vector.tensor_scalar_add(stde, stde, 1e-5)
        nc.vector.reciprocal(scv[:, 0:BPC], stde)  # rstd
        nc.vector.tensor_tensor(out=mr, in0=mean, in1=scv[:, 0:BPC], op=ALU.mult)
        nc.vector.tensor_scalar_mul(scv[:, BPC : 2 * BPC], mr, -1.0)
        # broadcast [rstd, -m*rstd] to all 128 partitions
        bc_ps = ppu.tile([128, 2 * BPC], F32, tag="pu")
        nc.tensor.matmul(bc_ps[:], ones_row[:], scv[:], start=True, stop=True)
        bc = spool.tile([128, 2 * BPC], F32, name="bc")
        nc.vector.tensor_copy(bc[:], bc_ps[:])

        ddscr = spool.tile([128, 8 * PS], BF16, name="ddscr")  # Square scratch

        state = {}

        def stage_ty(b):
            rstd_b = bc[:, b : b + 1]
            nm_b = bc[:, BPC + b : BPC + b + 1]
            # normalize in token layout, then DMA-transpose to patches^T:
            # the XBAR transpose maps out[p, k, u] = in[u, 128k+p], so the
            # normalized copy lives in a 4x-padded layout (cols 128k+ps)
            xcn = qpool.tile([128, 8 * 128], BF16, name=f"xcn{b}", tag="xcn")
            nc.gpsimd.memset(
                xcn[:].rearrange("u (k w) -> u k w", w=128)[:, :, PS:128], 0.0
            )
            nc.vector.tensor_scalar(
                out=xcn[:].rearrange("u (k w) -> u k w", w=128)[:, :, 0:PS],
                in0=xc[b][:].rearrange("u (k p) -> u k p", p=PS),
                scalar1=rstd_b,
                scalar2=nm_b,
                op0=ALU.mult,
                op1=ALU.add,
            )
            xnt = qpool.tile([PS, T], BF16, name=f"xnt{b}", tag="xnt")
            (nc.sync if b % 2 == 0 else nc.scalar).dma_start_transpose(
                xnt[:].rearrange("p (k u) -> p k u", k=8), xcn[:]
            )
            # Y = A^T X [32, 1024]
            yb = ypool.tile([PS, T], BF16, name=f"y{b}", tag="y")
            for h in range(2):
                y_ps = pxty.tile([128, 512], F32, tag="xty", name=f"yp{b}{h}")
                nc.tensor.matmul(
                    y_ps[0:PS, :],
                    a_mat[:],
                    xnt[:, 512 * h : 512 * (h + 1)],
                    start=True,
                    stop=True,
                )
                if h == 0:
                    nc.scalar.copy(yb[:, 0:512], y_ps[0:PS, :])
                else:
                    nc.vector.tensor_copy(yb[:, 512:1024], y_ps[0:PS, :])
            # shifted normalized target patches (token-major)
            xn2 = qpool.tile([128, 256], BF16, name=f"xn2{b}", tag="xn2")
            nc.gpsimd.tensor_scalar(
                out=xn2[:],
                in0=xc2[b][:],
                scalar1=rstd_b,
                scalar2=nm_b,
                op0=ALU.mult,
                op1=ALU.add,
            )
            vw = vpool.tile([128, 8 * (PS + 1)], BF16, name=f"vw{b}", tag="vw")
            nc.gpsimd.memset(
                vw[:].rearrange("u (j e) -> u j e", e=PS + 1)[:, :, PS : PS + 1],
                1.0,
            )
            if has_bias:
                # per-s logit bias gT = xnt^T w_kb -> exp bias tiles
                # (decoupled so exps only wait on their own score tile)
                g_ps = pxty.tile([128, 512], F32, tag="xty", name=f"g{b}")
                for j in range(8):
                    nc.tensor.matmul(
                        g_ps[:, j : j + 1],
                        xnt[:, 128 * j : 128 * (j + 1)],
                        wkb[:],
                        start=True,
                        stop=True,
                    )
                bias_d = spool.tile([128, 8], F32, name=f"bd{b}", tag="biasd")
                bias_s = spool.tile([128, 8], F32, name=f"bs{b}", tag="biass")
                nc.vector.tensor_scalar(
                    out=bias_d[:],
                    in0=g_ps[:, 0:8],
                    scalar1=K1 * SCALE,
                    scalar2=B0,
                    op0=ALU.mult,
                    op1=ALU.add,
                )
                nc.vector.tensor_scalar_mul(bias_s[:], g_ps[:, 0:8], SCALE)
            else:
                bias_d = bias_s = None
            state[b] = dict(
                xnt=xnt, yb=yb, xn2=xn2, vw=vw, bias_d=bias_d, bias_s=bias_s
            )

        def stage_scores(b, pv_prev=None):
            st = state[b]
            xnt, yb, vw = st["xnt"], st["yb"], st["vw"]
            vwg_ps = ppu.tile([128, 8 * (PS + 1)], F32, tag="pu", name=f"vwg{b}")
            bias_d, bias_s = st["bias_d"], st["bias_s"]
            et = epool.tile([128, 9216], BF16, name=f"et{b}", tag="et")
            st.update(et=et)
            score_tiles = {}

            def emit_scores(j):
                lhs = xnt[:, 128 * j : 128 * (j + 1)]
                nc.tensor.matmul(
                    vwg_ps[:, j * (PS + 1) : (j + 1) * (PS + 1)],
                    lhs,
                    mvg[:],
                    start=True,
                    stop=True,
                )
                s_ps = pscore.tile([128, 1024], F32, tag="sc", name=f"s{b}_{j}")
                score_tiles[j] = s_ps
                for c in range(2):
                    lo = max(512 * c, 128 * j)
                    hi = 512 * (c + 1)
                    if lo >= hi:
                        continue
                    nc.tensor.matmul(
                        s_ps[:, lo:hi], lhs, yb[:, lo:hi], start=True, stop=True
                    )

            def emit_vw_evac(half):
                # add c_v + cast to bf16 for 4 js; exp biases from gT col
                j0 = 4 * half
                vsl = slice(j0 * (PS + 1), (j0 + 4) * (PS + 1))
                pv_v = vwg_ps[:, vsl].rearrange("u (j e) -> u j e", e=PS + 1)
                vw_v = vw[:, vsl].rearrange("u (j e) -> u j e", e=PS + 1)
                nc.vector.tensor_tensor(
                    out=vw_v[:, :, 0:PS],
                    in0=pv_v[:, :, 0:PS],
                    in1=cv_rep[:]
                    .rearrange("u (o e) -> u o e", o=1)
                    .broadcast_to((128, 4, PS)),
                    op=ALU.add,
                )

            def emit_exp(j):
                s_ps = score_tiles.pop(j)
                src = s_ps[:, 128 * j : T]
                dst = et[:, 1024 * j + 128 * j : 1024 * (j + 1)]
                if j in DVE_EXP:
                    nc.vector.tensor_scalar(
                        out=dst.bitcast(I16),
                        in0=src,
                        scalar1=K1 * SCALE,
                        scalar2=bias_d[:, j : j + 1] if has_bias else B0,
                        op0=ALU.mult,
                        op1=ALU.add,
                    )
                else:
                    nc.scalar.activation(
                        dst,
                        src,
                        AF.Exp,
                        scale=SCALE,
                        bias=bias_s[:, j : j + 1] if has_bias else 0.0,
                    )

            def pv_group(tcn):
                if pv_prev is None:
                    return
                pst = state[pv_prev]
                pet, pvw = pst["et"], pst["vw"]
                for j in range(tcn + 1):
                    col = 1024 * j + 128 * tcn
                    nc.tensor.matmul(
                        pst["puT_ps"][:, tcn * (PS + 1) : (tcn + 1) * (PS + 1)],
                        pet[:, col : col + 128],
                        pvw[:, j * (PS + 1) : (j + 1) * (PS + 1)],
                        start=(j == 0),
                        stop=(j == tcn),
                    )

            if pv_prev is not None:
                state[pv_prev]["puT_ps"] = ppu.tile(
                    [128, 8 * (PS + 1)], F32, tag="pu", name=f"puT{pv_prev}"
                )
            for j in range(4):
                emit_scores(j)
                pv_group(j)
            emit_vw_evac(0)
            for j in range(4):
                emit_exp(j)
            for j in range(4, 8):
                emit_scores(j)
                pv_group(j)
            emit_vw_evac(1)
            for j in range(4, 8):
                emit_exp(j)
            # diag triu masks for THIS batch's et (stride 1152), split D/G
            vfull = et[:, 0 : 8 * 1152].rearrange("u (j w) -> u j w", w=1152)[
                :, :, 0:128
            ]
            tri4 = (
                triu_b[:]
                .rearrange("u (o w) -> u o w", o=1)
                .broadcast_to((128, 4, 128))
            )
            nc.vector.tensor_tensor(
                out=vfull[:, 0:4], in0=vfull[:, 0:4], in1=tri4, op=ALU.mult
            )
            nc.gpsimd.tensor_tensor(
                out=vfull[:, 4:8], in0=vfull[:, 4:8], in1=tri4, op=ALU.mult
            )

        def stage_pv_alone(b):
            st = state[b]
            et, vw = st["et"], st["vw"]
            st["puT_ps"] = ppu.tile(
                [128, 8 * (PS + 1)], F32, tag="pu", name=f"puT{b}"
            )
            for tcn in range(8):
                for j in range(tcn + 1):
                    col = 1024 * j + 128 * tcn
                    nc.tensor.matmul(
                        st["puT_ps"][:, tcn * (PS + 1) : (tcn + 1) * (PS + 1)],
                        et[:, col : col + 128],
                        vw[:, j * (PS + 1) : (j + 1) * (PS + 1)],
                        start=(j == 0),
                        stop=(j == tcn),
                    )

        def stage_epi(b):
            st = state[b]
            xn2 = st["xn2"]
            puT_ps = st["puT_ps"]
            # evacuate puT to SBUF immediately (frees the psum bank), then
            # normalize+subtract+square
            puT_sb = lpool.tile([128, 8 * (PS + 1)], F32, name=f"pus{b}", tag="pus")
            nc.vector.tensor_copy(puT_sb[:], puT_ps[:])
            rcol = lpool.tile([128, 8], F32, name=f"rc{b}", tag="rc")
            nc.vector.reciprocal(
                rcol[:],
                puT_sb[:].rearrange("u (c e) -> u c e", e=PS + 1)[
                    :, :, PS : PS + 1
                ].rearrange("u c o -> u (c o)"),
            )
            rexp = lpool.tile([128, 8 * PS], BF16, name=f"rx{b}", tag="rx")
            nc.vector.tensor_copy(
                rexp[:].rearrange("u (c p) -> u c p", p=PS),
                rcol[:].rearrange("u (c o) -> u c o", o=1).broadcast_to(
                    (128, 8, PS)
                ),
            )
            dd = lpool.tile([128, 8 * PS], BF16, name=f"dd{b}", tag="dd")
            nc.vector.tensor_tensor(
                out=dd[:].rearrange("u (c p) -> u c p", p=PS),
                in0=puT_sb[:].rearrange("u (c e) -> u c e", e=PS + 1)[
                    :, :, 0:PS
                ],
                in1=rexp[:].rearrange("u (c p) -> u c p", p=PS),
                op=ALU.mult,
            )
            nc.gpsimd.tensor_tensor(
                out=dd[:], in0=dd[:], in1=xn2[:], op=ALU.subtract
            )
            nc.vector.tensor_scalar(  # exclude t=1023
                out=dd[:, 7 * PS : 8 * PS],
                in0=dd[:, 7 * PS : 8 * PS],
                scalar1=lmask[:],
                scalar2=None,
                op0=ALU.mult,
            )
            nc.scalar.activation(
                ddscr[:], dd[:], AF.Square, accum_out=lp[:, b : b + 1]
            )

        # software pipeline: batch b+1's transposes/Y/scores fill the PE
        # while batch b's exps drain and its PV+epilogue wait on them
        stage_ty(0)
        stage_scores(0)
        for b in range(1, BPC):
            stage_ty(b)
            stage_scores(b, pv_prev=b - 1)
            stage_epi(b - 1)
        stage_pv_alone(BPC - 1)
        stage_epi(BPC - 1)

        # ---- final: total partial over batches & partitions ----
        lsum = spool.tile([128, 1], F32, name="lsum")
        nc.vector.tensor_reduce(lsum[:], lp[:], axis=AX.X, op=ALU.add)
        tot_ps2 = ppu.tile([1, 1], F32, tag="pu")
        nc.tensor.matmul(tot_ps2[:], ones_col[:], lsum[:], start=True, stop=True)
        out_sb = spool.tile([1, 1], F32, name="outsb")
        nc.vector.tensor_copy(out_sb[:], tot_ps2[:])
        nc.gpsimd.dma_start(out_d.ap()[:], out_sb[:])

    if postprocess:
        split_excess_waits(nc)
        dedupe_ldweights(nc)
    return nc


_program_cache = {}


def _get_program(has_bias=False):
    key = f"nc{int(has_bias)}"
    if key not in _program_cache:
        _program_cache[key] = build_program(has_bias=has_bias)
    return _program_cache[key]


def make_in_maps(x, W_proj, b_proj, W_qkv, b_qkv, W_out, b_out, W_head, b_head):
    import ml_dtypes

    f8 = np.float64
    w_eff = W_proj.astype(f8) @ W_qkv.astype(f8)  # [32, 768]
    b_eff = b_proj.astype(f8) @ W_qkv.astype(f8) + b_qkv.astype(f8)  # [768]
    Wq, Wk, Wv = w_eff[:, 0:D], w_eff[:, D : 2 * D], w_eff[:, 2 * D : 3 * D]
    bq = b_eff[0:D]
    a_mat = Wq @ Wk.T  # [32, 32]; device computes Y = a_mat^T @ X
    w_kb = Wk @ bq  # [32] per-s logit bias
    w_oh = W_out.astype(f8) @ W_head.astype(f8)  # [256, 32]
    b_oh = b_out.astype(f8) @ W_head.astype(f8) + b_head.astype(f8)  # [32]
    m_v = Wv @ w_oh  # [32, 32]
    c_v = b_eff[2 * D : 3 * D] @ w_oh + b_oh  # [32]

    a_b = np.ascontiguousarray(a_mat.astype(ml_dtypes.bfloat16))
    mvg_b = np.ascontiguousarray(
        np.concatenate([m_v, w_kb[:, None]], axis=1).astype(ml_dtypes.bfloat16)
    )
    cv_rep = np.ascontiguousarray(
        np.tile(c_v.astype(np.float32)[None, :], (128, 1))
    )
    wkb_b = np.ascontiguousarray(w_kb[:, None].astype(ml_dtypes.bfloat16))

    in_maps = []
    for core in range(N_CORES):
        xs = np.ascontiguousarray(x[core * BPC : (core + 1) * BPC])
        in_maps.append(
            {"x": xs, "a_mat": a_b, "mvg": mvg_b, "cv_rep": cv_rep, "wkb": wkb_b}
        )
    return in_maps


def kernel(**inputs) -> np.ndarray:
    inputs = {k: np.asarray(v) for k, v in inputs.items()}
    has_bias = any(
        float(np.abs(np.asarray(inputs[k])).max()) != 0.0
        for k in ("b_proj", "b_qkv")
    )
    nc = _get_program(has_bias)
    in_maps = make_in_maps(**inputs)
    res = run_bass_kernel_spmd(nc, in_maps, core_ids=list(range(N_CORES)))
    total = sum(float(res.results[i]["loss_partial"][0, 0]) for i in range(N_CORES))
    loss = total / (B * (T - 1) * PS)
    return np.float32(loss)


if __name__ == "__main__":
    rng = np.random.default_rng(0)
    ins = {
        "x": rng.standard_normal((B, L)).astype(np.float32),
        "W_proj": (rng.standard_normal((PS, D)) / math.sqrt(PS)).astype(np.float32),
        "b_proj": np.zeros(D, np.float32),
        "W_qkv": (rng.standard_normal((D, 3 * D)) / math.sqrt(D)).astype(np.float32),
        "b_qkv": np.zeros(3 * D, np.float32),
        "W_out": (rng.standard_normal((D, D)) / math.sqrt(D)).astype(np.float32),
        "b_out": np.zeros(D, np.float32),
        "W_head": (rng.standard_normal((D, PS)) / math.sqrt(D)).astype(np.float32),
        "b_head": np.zeros(PS, np.float32),
    }
    got = kernel(**ins)
    print("kernel loss:", got)


# revision 33
# speedup vs baseline: 1.0649x; 1.0649x over previous
"""Trainium2 Bass kernel for nn_AttentionModel (patch-transformer + MSE loss).

Model (per batch element b of B=32):
    x[b] : [L=32768] --instance-norm--> patches [T=1024, PS=32]
    h    = patches @ W_proj + b_proj                  [T, 256]
    qkv  = h @ W_qkv + b_qkv ;  q,k,v = split(qkv)    [T, 256] each
    attn = softmax(causal(q k^T / 16))                [T, T]
    out  = (attn @ v) @ W_out + b_out                 [T, 256]
    pred = out @ W_head + b_head                      [T, PS]
    loss = mean((pred[:, :-1] - patches[:, 1:])**2)   scalar

Sharding: data-parallel over batch, 4 batch elements per core x 8 cores.
Each core computes a partial sum-of-squares; host combines.

v2.1 design (numpy prototype: proto_v2.py):
  - logits factor through patch space: scores[t,s] = x_t^T A x_s + g_s with
    A = Wq_eff Wk_eff^T [32,32]; the per-t bias constant is dropped
    (softmax-invariant), the per-s part g_s = (Wk b_q)^T x_s rides as a
    per-partition bias of the exp.
  - X = patches^T [32, T] bf16 normalized; Y = A^T X once [32, T];
    scores^T[s,t] computed per s-tile j as ONE wide matmul (N<=1024 bf16),
    causally trimmed.
  - exp is split between ScalarE (ACT exp, per-partition bias) and DVE
    (int16 bit-trick exp writing bf16 bits; its systematic error cancels
    in the softmax ratio -- validated 4.5e-6 rel err in proto).
  - PV runs TRANSPOSED: puT[tc] = sum_j et(j,tc)^T vw_j with
    vw = [X^T M_v + 1 c_v^T | ones], so the softmax denominator lands
    per-PARTITION (puT col 32) and normalize+loss is all per-partition:
    reciprocal [128,8] -> fused (puT*r - xn2) scalar_tensor_tensor ->
    Square+accum loss.
  - the shifted target patches xn2 (= patch[t+1] token-major) come from a
    second strided DMA of x (no transposes, no partition shifts).
  - NOTE row-tiled matmuls (tile_position with row base != 0) pass
    compile+sim but hang this HW stack when >1 issue back-to-back; only
    plain/col tiling is used.
"""

import math
import os

import numpy as np

import concourse.bass as bass
import concourse.mybir as mybir
import concourse.tile as tile
from concourse.bass_utils import run_bass_kernel_spmd
from concourse.masks import make_identity, make_upper_triangular
from concourse.vector_clock import ScopedClock

F32 = mybir.dt.float32
BF16 = mybir.dt.bfloat16
I16 = mybir.dt.int16
AX = mybir.AxisListType
ALU = mybir.AluOpType
AF = mybir.ActivationFunctionType

N_CORES = 8
B = 32
L = 32768
PS = 32
D = 256
T = L // PS  # 1024
BPC = B // N_CORES  # batch elements per core = 4
SCALE = 1.0 / math.sqrt(D)  # 1/16
K1 = 128.0 / math.log(2.0)  # bf16 exponent scale for bit-trick exp
B0 = 127.0 * 128.0 - 0.0579 * 128.0  # bias incl. mantissa-centering corr

# j-tiles whose exp runs on DVE (bit-trick); rest on ScalarE ACT exp.
DVE_EXP = {0, 1}


class SplitDrainTileContext(tile.TileContext):
    """TileContext whose final drain splits sem waits across multiple drain
    instructions -- this walrus rejects >1 sync wait per instruction."""

    def _drain_and_barrier(self, tick_clock, wait_clock):
        probe = mybir.InstDrain(name=f"I-{self.nc.next_id()}", ins=[], outs=[])
        probe.engine = mybir.EngineType.SP
        wait_clock.add_sem_waits(probe, ScopedClock({None: tick_clock.global_clock}))
        waits = list(probe.sync_info.on_wait) if probe.sync_info else []
        assert self.sems is not None
        handles = {h.num: h for h in self.sems.allocated().values()}
        if not waits:
            self.nc.sync.drain()
        for w in waits:
            d = self.nc.sync.drain()
            d.wait_op(handles[w.id], w.wait_value, "sem-ge", check=False)
        self.nc.all_engine_barrier()
        popped = self.nc._tile_sem_poison_stack.pop()
        assert popped is self._sem_poison
        self.nc.clear_and_free_semaphores(list(self.sems.allocated().values()))
        self.nc.all_engine_barrier()


def split_excess_waits(nc, max_waits=1):
    """This walrus rejects instructions carrying more than one sync wait.
    Hoist extra waits onto the immediately preceding same-engine
    instruction when that instruction signals nothing, else insert a
    wait-only drain."""
    for f in nc.m.functions:
        for blk in f.blocks:
            insts = list(blk.instructions)
            out = []
            prev_by_engine = {}
            changed = False
            for inst in insts:
                si = inst.sync_info
                waits = list(si.on_wait) if si else []
                if len(waits) > max_waits:
                    changed = True
                    extra, keep = waits[:-max_waits], waits[-max_waits:]
                    remaining = []
                    prev = prev_by_engine.get(str(inst.engine))
                    for w in extra:
                        psi = prev.sync_info if prev is not None else None
                        if prev is not None and (
                            psi is None
                            or (len(psi.on_wait) == 0 and len(psi.on_update) == 0)
                        ):
                            prev.sync_info = mybir.SyncInfo(on_wait=[w], on_update=[])
                            prev = None  # one hoist per predecessor
                        else:
                            remaining.append(w)
                    for w in remaining:
                        dr = mybir.InstDrain(name=f"I-{nc.next_id()}", ins=[], outs=[])
                        dr.engine = inst.engine
                        dr.sync_info = mybir.SyncInfo(on_wait=[w], on_update=[])
                        out.append(dr)
                    inst.sync_info = mybir.SyncInfo(
                        on_wait=keep, on_update=list(si.on_update)
                    )
                out.append(inst)
                prev_by_engine[str(inst.engine)] = inst
            if changed:
                blk.instructions = out


def dedupe_ldweights(nc):
    """Drop an InstLdweights whose operand AP AND tile_position are
    byte-identical to the immediately preceding PE instruction's
    InstLdweights -- the stationary operand is still loaded. Only legal
    when the elided load carries no sync actions."""
    for f in nc.m.functions:
        for blk in f.blocks:
            insts = list(blk.instructions)
            out = []
            last_pe_ldw_key = None
            changed = False
            for inst in insts:
                if str(inst.engine) != "EngineType.PE":
                    out.append(inst)
                    continue
                tname = type(inst).__name__
                if tname == "InstLdweights":
                    si = inst.sync_info
                    has_sync = si and (len(si.on_wait) or len(si.on_update))
                    try:
                        key = (
                            str(inst.ins[0]),
                            str(getattr(inst, "tile_position", None)),
                        )
                    except Exception:
                        key = None
                    if key is not None and key == last_pe_ldw_key and not has_sync:
                        changed = True
                        continue  # elide duplicate load
                    last_pe_ldw_key = key
                    out.append(inst)
                else:
                    if tname == "InstMatmult":
                        # transpose-mode matmuls reload the array themselves
                        if getattr(inst, "is_transpose", None):
                            last_pe_ldw_key = None
                    else:
                        last_pe_ldw_key = None
                    out.append(inst)
            if changed:
                blk.instructions = out


def build_program(postprocess=True, has_bias=False):
    nc = bass.Bass("TRN2", target_bir_lowering=False, debug=False, num_devices=N_CORES)

    x_d = nc.dram_tensor("x", [BPC, L], F32, kind="ExternalInput")
    a_d = nc.dram_tensor("a_mat", [PS, PS], BF16, kind="ExternalInput")
    mvg_d = nc.dram_tensor("mvg", [PS, PS + 1], BF16, kind="ExternalInput")
    cv_d = nc.dram_tensor("cv_rep", [128, PS], F32, kind="ExternalInput")
    wkb_d = nc.dram_tensor("wkb", [PS, 1], BF16, kind="ExternalInput")
    out_d = nc.dram_tensor("loss_partial", [1, 1], F32, kind="ExternalOutput")

    from contextlib import ExitStack

    with SplitDrainTileContext(nc) as tc, ExitStack() as ctx:
        cpool = ctx.enter_context(tc.tile_pool(name="consts", bufs=1))
        # PSUM pools (8 banks x 2KB/partition): pscore 3x2 banks + ppu 2x1
        pscore = ctx.enter_context(tc.tile_pool(name="pscore", bufs=2, space="PSUM"))
        pxty = ctx.enter_context(tc.tile_pool(name="pxty", bufs=2, space="PSUM"))
        ppu = ctx.enter_context(tc.tile_pool(name="ppu", bufs=2, space="PSUM"))
        # SBUF pools
        xpool = ctx.enter_context(tc.tile_pool(name="xc", bufs=4))
        x2pool = ctx.enter_context(tc.tile_pool(name="xc2", bufs=4))
        qpool = ctx.enter_context(tc.tile_pool(name="xnt", bufs=4))
        ypool = ctx.enter_context(tc.tile_pool(name="y", bufs=3))
        vpool = ctx.enter_context(tc.tile_pool(name="vw", bufs=3))
        epool = ctx.enter_context(tc.tile_pool(name="et", bufs=3))
        spool = ctx.enter_context(tc.tile_pool(name="small", bufs=6))
        lpool = ctx.enter_context(tc.tile_pool(name="loss", bufs=4))

        # ---- constants ----
        ident_f = cpool.tile([128, 128], F32)
        make_identity(nc, ident_f[:])
        triu_b = cpool.tile([128, 128], BF16)
        make_upper_triangular(nc, triu_b[:], val=1.0, diag=True)
        ones_col = cpool.tile([128, 1], F32)
        nc.vector.memset(ones_col[:], 1.0)
        ones_row = cpool.tile([1, 128], F32)
        nc.vector.memset(ones_row[:], 1.0)

        a_mat = cpool.tile([PS, PS], BF16)
        nc.gpsimd.dma_start(a_mat[:], a_d.ap()[:])
        mvg = cpool.tile([PS, PS + 1], BF16)
        nc.gpsimd.dma_start(mvg[:], mvg_d.ap()[:])
        cv_rep = cpool.tile([128, PS], F32)
        nc.gpsimd.dma_start(cv_rep[:], cv_d.ap()[:])
        wkb = cpool.tile([PS, 1], BF16)
        nc.gpsimd.dma_start(wkb[:], wkb_d.ap()[:])

        lp = cpool.tile([128, BPC], F32)  # per-batch loss partials
        # last-partition mask: zeroes the t=1023 dd column (excluded by
        # pred[:, :-1]; its xn2 is garbage)
        lmask = cpool.tile([128, 1], F32)
        nc.vector.tensor_scalar(
            out=lmask[:],
            in0=ident_f[:, 127:128],
            scalar1=-1.0,
            scalar2=1.0,
            op0=ALU.mult,
            op1=ALU.add,
        )

        # PE warm-up: HAM holds PE at 1.2 GHz until ~3.4us sustained
        # activity; burn dummy matmuls while the DMAs + stats run.
        warm_ps = pscore.tile([128, 1024], F32, tag="sc")
        for _ in range(42):
            nc.tensor.matmul(
                warm_ps[:, 0:128], triu_b[:], triu_b[:], start=True, stop=True
            )

        # ---- load x and x-shifted-by-one-patch; stats for all batches ----
        xc = []
        xc2 = []
        for b in range(BPC):
            t_ = xpool.tile([128, L // 128], F32, name=f"xc{b}", tag="xc")
            # partition u, free (k, ps) <- x[b, (128k+u)*32 + ps]
            nc.sync.dma_start(
                t_[:].rearrange("u (k ps) -> u k ps", ps=PS),
                x_d.ap()[b].rearrange("(k u ps) -> u k ps", u=128, ps=PS),
            )
            xc.append(t_)
            t2 = x2pool.tile([128, L // 128], F32, name=f"xc2{b}", tag="xc2")
            # the DMA leaves [127, 224:256] unwritten (no patch 1024); zero it
            # first so the masked-out t=1023 lane can't be NaN garbage
            nc.gpsimd.memset(t2[:, 7 * PS : 8 * PS], 0.0)
            # token-major target patches, shifted one patch
            nc.sync.dma_start(
                t2[:, 0 : 7 * PS].rearrange("u (c p) -> u c p", p=PS),
                x_d.ap()[b, PS : PS + 7 * 128 * PS].rearrange(
                    "(c u p) -> u c p", u=128, p=PS
                ),
            )
            nc.sync.dma_start(
                t2[0:127, 7 * PS : 8 * PS],
                x_d.ap()[b, PS + 7 * 128 * PS : L].rearrange("(u p) -> u p", p=PS),
            )
            xc2.append(t2)

        sums = spool.tile([128, 2 * BPC], F32, name="sums")
        sqscr = spool.tile([128, L // 128], F32, name="sqscr")
        for b in range(BPC):
            nc.vector.tensor_reduce(
                sums[:, b : b + 1], xc[b][:], axis=AX.X, op=ALU.add
            )
            nc.scalar.activation(
                sqscr[:],
                xc[b][:],
                AF.Square,
                accum_out=sums[:, BPC + b : BPC + b + 1],
            )
        tot_ps = ppu.tile([1, 2 * BPC], F32, tag="pu")
        nc.tensor.matmul(tot_ps[:], ones_col[:], sums[:], start=True, stop=True)
        tot = spool.tile([1, 2 * BPC], F32, name="tot")
        nc.vector.tensor_copy(tot[:], tot_ps[:])

        # vectorized stats chain over the 4 batches
        sc = spool.tile([1, 6 * BPC], F32, name="sc")
        mean = sc[:, 0:BPC]
        sm = sc[:, BPC : 2 * BPC]
        varr = sc[:, 2 * BPC : 3 * BPC]
        lnv = sc[:, 3 * BPC : 4 * BPC]
        stde = sc[:, 4 * BPC : 5 * BPC]
        mr = sc[:, 5 * BPC : 6 * BPC]
        scv = spool.tile([1, 2 * BPC], F32, name="scv")  # [rstd x4, -m*rstd x4]
        nc.scalar.mul(mean, tot[:, 0:BPC], 1.0 / L)
        nc.vector.tensor_tensor(out=sm, in0=tot[:, 0:BPC], in1=mean, op=ALU.mult)
        nc.vector.tensor_tensor(
            out=varr, in0=tot[:, BPC : 2 * BPC], in1=sm, op=ALU.subtract
        )
        nc.scalar.activation(lnv, varr, AF.Ln, scale=1.0 / (L - 1))
        nc.scalar.activation(stde, lnv, AF.Exp, scale=0.5)  # std
        nc.vector.tensor_scalar_add(stde, stde, 1e-5)
        nc.vector.reciprocal(scv[:, 0:BPC], stde)  # rstd
        nc.vector.tensor_tensor(out=mr, in0=mean, in1=scv[:, 0:BPC], op=ALU.mult)
        nc.vector.tensor_scalar_mul(scv[:, BPC : 2 * BPC], mr, -1.0)
        # broadcast [rstd, -m*rstd] to all 128 partitions
        bc_ps = ppu.tile([128, 2 * BPC], F32, tag="pu")
        nc.tensor.matmul(bc_ps[:], ones_row[:], scv[:], start=True, stop=True)
        bc = spool.tile([128, 2 * BPC], F32, name="bc")
        nc.vector.tensor_copy(bc[:], bc_ps[:])

        ddscr = spool.tile([128, 8 * PS], BF16, name="ddscr")  # Square scratch

        state = {}

        def stage_ty(b):
            rstd_b = bc[:, b : b + 1]
            nm_b = bc[:, BPC + b : BPC + b + 1]
            # normalize in token layout, then DMA-transpose to patches^T:
            # the XBAR transpose maps out[p, k, u] = in[u, 128k+p], so the
            # normalized copy lives in a 4x-padded layout (cols 128k+ps)
            xcn = qpool.tile([128, 8 * 128], BF16, name=f"xcn{b}", tag="xcn")
            nc.gpsimd.memset(
                xcn[:].rearrange("u (k w) -> u k w", w=128)[:, :, PS:128], 0.0
            )
            nc.vector.tensor_scalar(
                out=xcn[:].rearrange("u (k w) -> u k w", w=128)[:, :, 0:PS],
                in0=xc[b][:].rearrange("u (k p) -> u k p", p=PS),
                scalar1=rstd_b,
                scalar2=nm_b,
                op0=ALU.mult,
                op1=ALU.add,
            )
            xnt = qpool.tile([PS, T], BF16, name=f"xnt{b}", tag="xnt")
            nc.scalar.dma_start_transpose(
                xnt[:].rearrange("p (k u) -> p k u", k=8), xcn[:]
            )
            # Y = A^T X [32, 1024]
            yb = ypool.tile([PS, T], BF16, name=f"y{b}", tag="y")
            for h in range(2):
                y_ps = pxty.tile([128, 512], F32, tag="xty", name=f"yp{b}{h}")
                nc.tensor.matmul(
                    y_ps[0:PS, :],
                    a_mat[:],
                    xnt[:, 512 * h : 512 * (h + 1)],
                    start=True,
                    stop=True,
                )
                if h == 0:
                    nc.scalar.copy(yb[:, 0:512], y_ps[0:PS, :])
                else:
                    nc.vector.tensor_copy(yb[:, 512:1024], y_ps[0:PS, :])
            # shifted normalized target patches (token-major)
            xn2 = qpool.tile([128, 256], BF16, name=f"xn2{b}", tag="xn2")
            nc.gpsimd.tensor_scalar(
                out=xn2[:],
                in0=xc2[b][:],
                scalar1=rstd_b,
                scalar2=nm_b,
                op0=ALU.mult,
                op1=ALU.add,
            )
            vw = vpool.tile([128, 8 * (PS + 1)], BF16, name=f"vw{b}", tag="vw")
            nc.gpsimd.memset(
                vw[:].rearrange("u (j e) -> u j e", e=PS + 1)[:, :, PS : PS + 1],
                1.0,
            )
            if has_bias:
                # per-s logit bias gT = xnt^T w_kb -> exp bias tiles
                # (decoupled so exps only wait on their own score tile)
                g_ps = pxty.tile([128, 512], F32, tag="xty", name=f"g{b}")
                for j in range(8):
                    nc.tensor.matmul(
                        g_ps[:, j : j + 1],
                        xnt[:, 128 * j : 128 * (j + 1)],
                        wkb[:],
                        start=True,
                        stop=True,
                    )
                bias_d = spool.tile([128, 8], F32, name=f"bd{b}", tag="biasd")
                bias_s = spool.tile([128, 8], F32, name=f"bs{b}", tag="biass")
                nc.vector.tensor_scalar(
                    out=bias_d[:],
                    in0=g_ps[:, 0:8],
                    scalar1=K1 * SCALE,
                    scalar2=B0,
                    op0=ALU.mult,
                    op1=ALU.add,
                )
                nc.vector.tensor_scalar_mul(bias_s[:], g_ps[:, 0:8], SCALE)
            else:
                bias_d = bias_s = None
            state[b] = dict(
                xnt=xnt, yb=yb, xn2=xn2, vw=vw, bias_d=bias_d, bias_s=bias_s
            )

        def stage_scores(b, pv_prev=None):
            st = state[b]
            xnt, yb, vw = st["xnt"], st["yb"], st["vw"]
            vwg_ps = ppu.tile([128, 8 * (PS + 1)], F32, tag="pu", name=f"vwg{b}")
            bias_d, bias_s = st["bias_d"], st["bias_s"]
            et = epool.tile([128, 9216], BF16, name=f"et{b}", tag="et")
            st.update(et=et)
            score_tiles = {}

            def emit_scores(j):
                lhs = xnt[:, 128 * j : 128 * (j + 1)]
                nc.tensor.matmul(
                    vwg_ps[:, j * (PS + 1) : (j + 1) * (PS + 1)],
                    lhs,
                    mvg[:],
                    start=True,
                    stop=True,
                )
                s_ps = pscore.tile([128, 1024], F32, tag="sc", name=f"s{b}_{j}")
                score_tiles[j] = s_ps
                for c in range(2):
                    lo = max(512 * c, 128 * j)
                    hi = 512 * (c + 1)
                    if lo >= hi:
                        continue
                    nc.tensor.matmul(
                        s_ps[:, lo:hi], lhs, yb[:, lo:hi], start=True, stop=True
                    )

            def emit_vw_evac(half):
                # add c_v + cast to bf16 for 4 js; exp biases from gT col
                j0 = 4 * half
                vsl = slice(j0 * (PS + 1), (j0 + 4) * (PS + 1))
                pv_v = vwg_ps[:, vsl].rearrange("u (j e) -> u j e", e=PS + 1)
                vw_v = vw[:, vsl].rearrange("u (j e) -> u j e", e=PS + 1)
                nc.vector.tensor_tensor(
                    out=vw_v[:, :, 0:PS],
                    in0=pv_v[:, :, 0:PS],
                    in1=cv_rep[:]
                    .rearrange("u (o e) -> u o e", o=1)
                    .broadcast_to((128, 4, PS)),
                    op=ALU.add,
                )

            def emit_exp(j):
                s_ps = score_tiles.pop(j)
                src = s_ps[:, 128 * j : T]
                dst = et[:, 1024 * j + 128 * j : 1024 * (j + 1)]
                if j in DVE_EXP:
                    nc.vector.tensor_scalar(
                        out=dst.bitcast(I16),
                        in0=src,
                        scalar1=K1 * SCALE,
                        scalar2=bias_d[:, j : j + 1] if has_bias else B0,
                        op0=ALU.mult,
                        op1=ALU.add,
                    )
                else:
                    nc.scalar.activation(
                        dst,
                        src,
                        AF.Exp,
                        scale=SCALE,
                        bias=bias_s[:, j : j + 1] if has_bias else 0.0,
                    )

            def pv_group(tcn):
                if pv_prev is None:
                    return
                pst = state[pv_prev]
                pet, pvw = pst["et"], pst["vw"]
                for j in range(tcn + 1):
                    col = 1024 * j + 128 * tcn
                    nc.tensor.matmul(
                        pst["puT_ps"][:, tcn * (PS + 1) : (tcn + 1) * (PS + 1)],
                        pet[:, col : col + 128],
                        pvw[:, j * (PS + 1) : (j + 1) * (PS + 1)],
                        start=(j == 0),
                        stop=(j == tcn),
                    )

            if pv_prev is not None:
                state[pv_prev]["puT_ps"] = ppu.tile(
                    [128, 8 * (PS + 1)], F32, tag="pu", name=f"puT{pv_prev}"
                )
            for j in range(4):
                emit_scores(j)
                pv_group(j)
            emit_vw_evac(0)
            for j in range(4):
                emit_exp(j)
            for j in range(4, 8):
                emit_scores(j)
                pv_group(j)
            emit_vw_evac(1)
            for j in range(4, 8):
                emit_exp(j)
            # diag triu masks for THIS batch's et (stride 1152), split D/G
            vfull = et[:, 0 : 8 * 1152].rearrange("u (j w) -> u j w", w=1152)[
                :, :, 0:128
            ]
            tri4 = (
                triu_b[:]
                .rearrange("u (o w) -> u o w", o=1)
                .broadcast_to((128, 4, 128))
            )
            nc.vector.tensor_tensor(
                out=vfull[:, 0:4], in0=vfull[:, 0:4], in1=tri4, op=ALU.mult
            )
            nc.gpsimd.tensor_tensor(
                out=vfull[:, 4:8], in0=vfull[:, 4:8], in1=tri4, op=ALU.mult
            )

        def stage_pv_alone(b):
            st = state[b]
            et, vw = st["et"], st["vw"]
            st["puT_ps"] = ppu.tile(
                [128, 8 * (PS + 1)], F32, tag="pu", name=f"puT{b}"
            )
            for tcn in range(8):
                for j in range(tcn + 1):
                    col = 1024 * j + 128 * tcn
                    nc.tensor.matmul(
                        st["puT_ps"][:, tcn * (PS + 1) : (tcn + 1) * (PS + 1)],
                        et[:, col : col + 128],
                        vw[:, j * (PS + 1) : (j + 1) * (PS + 1)],
                        start=(j == 0),
                        stop=(j == tcn),
                    )

        def stage_epi(b):
            st = state[b]
            xn2 = st["xn2"]
            puT_ps = st["puT_ps"]
            # evacuate puT to SBUF immediately (frees the psum bank), then
            # normalize+subtract+square
            puT_sb = lpool.tile([128, 8 * (PS + 1)], F32, name=f"pus{b}", tag="pus")
            nc.vector.tensor_copy(puT_sb[:], puT_ps[:])
            rcol = lpool.tile([128, 8], F32, name=f"rc{b}", tag="rc")
            nc.vector.reciprocal(
                rcol[:],
                puT_sb[:].rearrange("u (c e) -> u c e", e=PS + 1)[
                    :, :, PS : PS + 1
                ].rearrange("u c o -> u (c o)"),
            )
            rexp = lpool.tile([128, 8 * PS], BF16, name=f"rx{b}", tag="rx")
            nc.vector.tensor_copy(
                rexp[:].rearrange("u (c p) -> u c p", p=PS),
                rcol[:].rearrange("u (c o) -> u c o", o=1).broadcast_to(
                    (128, 8, PS)
                ),
            )
            dd = lpool.tile([128, 8 * PS], BF16, name=f"dd{b}", tag="dd")
            nc.vector.tensor_tensor(
                out=dd[:].rearrange("u (c p) -> u c p", p=PS),
                in0=puT_sb[:].rearrange("u (c e) -> u c e", e=PS + 1)[
                    :, :, 0:PS
                ],
                in1=rexp[:].rearrange("u (c p) -> u c p", p=PS),
                op=ALU.mult,
            )
            nc.gpsimd.tensor_tensor(
                out=dd[:], in0=dd[:], in1=xn2[:], op=ALU.subtract
            )
            nc.vector.tensor_scalar(  # exclude t=1023
                out=dd[:, 7 * PS : 8 * PS],
                in0=dd[:, 7 * PS : 8 * PS],
                scalar1=lmask[:],
                scalar2=None,
                op0=ALU.mult,
            )
            nc.scalar.activation(
                ddscr[:], dd[:], AF.Square, accum_out=lp[:, b : b + 1]
            )

        # software pipeline: batch b+1's transposes/Y/scores fill the PE
        # while batch b's exps drain and its PV+epilogue wait on them
        stage_ty(0)
        stage_scores(0)
        for b in range(1, BPC):
            stage_ty(b)
            stage_scores(b, pv_prev=b - 1)
            stage_epi(b - 1)
        stage_pv_alone(BPC - 1)
        stage_epi(BPC - 1)

        # ---- final: total partial over batches & partitions ----
        lsum = spool.tile([128, 1], F32, name="lsum")
        nc.vector.tensor_reduce(lsum[:], lp[:], axis=AX.X, op=ALU.add)
        tot_ps2 = ppu.tile([1, 1], F32, tag="pu")
        nc.tensor.matmul(tot_ps2[:], ones_col[:], lsum[:], start=True, stop=True)
        out_sb = spool.tile([1, 1], F32, name="outsb")
        nc.vector.tensor_copy(out_sb[:], tot_ps2[:])
        nc.gpsimd.dma_start(out_d.ap()[:], out_sb[:])

    if postprocess:
        split_excess_waits(nc)
        dedupe_ldweights(nc)
    return nc


_program_cache = {}


def _get_program(has_bias=False):
    key = f"nc{int(has_bias)}"
    if key not in _program_cache:
        _program_cache[key] = build_program(has_bias=has_bias)
    return _program_cache[key]


def make_in_maps(x, W_proj, b_proj, W_qkv, b_qkv, W_out, b_out, W_head, b_head):
    import ml_dtypes

    f8 = np.float64
    w_eff = W_proj.astype(f8) @ W_qkv.astype(f8)  # [32, 768]
    b_eff = b_proj.astype(f8) @ W_qkv.astype(f8) + b_qkv.astype(f8)  # [768]
    Wq, Wk, Wv = w_eff[:, 0:D], w_eff[:, D : 2 * D], w_eff[:, 2 * D : 3 * D]
    bq = b_eff[0:D]
    a_mat = Wq @ Wk.T  # [32, 32]; device computes Y = a_mat^T @ X
    w_kb = Wk @ bq  # [32] per-s logit bias
    w_oh = W_out.astype(f8) @ W_head.astype(f8)  # [256, 32]
    b_oh = b_out.astype(f8) @ W_head.astype(f8) + b_head.astype(f8)  # [32]
    m_v = Wv @ w_oh  # [32, 32]
    c_v = b_eff[2 * D : 3 * D] @ w_oh + b_oh  # [32]

    a_b = np.ascontiguousarray(a_mat.astype(ml_dtypes.bfloat16))
    mvg_b = np.ascontiguousarray(
        np.concatenate([m_v, w_kb[:, None]], axis=1).astype(ml_dtypes.bfloat16)
    )
    cv_rep = np.ascontiguousarray(
        np.tile(c_v.astype(np.float32)[None, :], (128, 1))
    )
    wkb_b = np.ascontiguousarray(w_kb[:, None].astype(ml_dtypes.bfloat16))

    in_maps = []
    for core in range(N_CORES):
        xs = np.ascontiguousarray(x[core * BPC : (core + 1) * BPC])
        in_maps.append(
            {"x": xs, "a_mat": a_b, "mvg": mvg_b, "cv_rep": cv_rep, "wkb": wkb_b}
        )
    return in_maps


def kernel(**inputs) -> np.ndarray:
    inputs = {k: np.asarray(v) for k, v in inputs.items()}
    has_bias = any(
        float(np.abs(np.asarray(inputs[k])).max()) != 0.0
        for k in ("b_proj", "b_qkv")
    )
    nc = _get_program(has_bias)
    in_maps = make_in_maps(**inputs)
    res = run_bass_kernel_spmd(nc, in_maps, core_ids=list(range(N_CORES)))
    total = sum(float(res.results[i]["loss_partial"][0, 0]) for i in range(N_CORES))
    loss = total / (B * (T - 1) * PS)
    return np.float32(loss)


if __name__ == "__main__":
    rng = np.random.default_rng(0)
    ins = {
        "x": rng.standard_normal((B, L)).astype(np.float32),
        "W_proj": (rng.standard_normal((PS, D)) / math.sqrt(PS)).astype(np.float32),
        "b_proj": np.zeros(D, np.float32),
        "W_qkv": (rng.standard_normal((D, 3 * D)) / math.sqrt(D)).astype(np.float32),
        "b_qkv": np.zeros(3 * D, np.float32),
        "W_out": (rng.standard_normal((D, D)) / math.sqrt(D)).astype(np.float32),
        "b_out": np.zeros(D, np.float32),
        "W_head": (rng.standard_normal((D, PS)) / math.sqrt(D)).astype(np.float32),
        "b_head": np.zeros(PS, np.float32),
    }
    got = kernel(**ins)
    print("kernel loss:", got)
